# revision 5
# baseline (speedup 1.0000x reference)
"""Backward_projection (FBP: ramp filter + backprojection) on 8 trn2 NeuronCores.

Device formulation (hand-written Bass/Tile kernel, one NEFF per core, SPMD):
  * The ramp filter (exact 183x183 circulant-section matmul, all scalar
    factors folded) is FOLDED into the backprojection matrix:
    out[b, p] = sum_d x[b, a, d] * Wt_a[d, p],  Wt_a = F @ tri(d - k[a, p]),
    so the whole FBP is one accumulation of 285 per-angle matmuls on the PE.
  * Angle-flip pairing: k_{284-a}(127-i, j) == k_a(i, j) exactly, so the
    weight tile of (angle a, image row i) also serves (284-a, 127-i); each
    matmul streams N=512 (256 batches x 2 pair-halves) through one
    stationary tile.  Angle 142 (theta = pi/2) is row-independent and is
    computed once and added in the epilogue.
  * K = 183 splits into chunks of 128 + 55; the 55-chunks of two adjacent
    pairs are packed into one [110, .] stationary tile (25% fewer streams).
  * Sharding: each core owns the symmetric image-row set
    [8c, 8c+8) u [120-8c, 128-8c) (16 rows, 2048 pixels) for all 256
    batches, accumulated over two 8-row PSUM sweeps.  x is uploaded
    batch-sharded (contiguous slices) and all-gathered on device.
  * Weights (~107MB/core bf16) are GENERATED ON DEVICE once from a small
    geometry table and stay resident; per call only the quantized sinogram
    crosses the (slow) host link and the bf16 image returns.

The wire uses int8 (validated against the reference: quantization noise
contributes ~1e-2 of the 2e-2 absmax-relative budget; the bf16 path at
~5e-3 is kept as an automatic fallback for inputs exceeding the int8
range).  Falls back to a host numba/numpy implementation if the device
path is unavailable.
"""

import numpy as np

# --- geometry constants (parallel_beam_geometry on a 128^2 grid) ---
N = 128
CELL = 40.0 / N
RHO = float(np.sqrt(2.0) * 20.0)
A = 285
D = 183
DC = 2.0 * RHO / D
PAD = 512
B = 256
NPAIR = 142
INT8_SCALE = 127.0 / 5.5
INT8_ABSMAX = 5.45  # inputs beyond this use the bf16 wire


def _filter_matrix():
    n = (np.fft.fftfreq(PAD) * PAD).astype(np.int64)
    h = np.zeros(PAD, np.float64)
    h[0] = 1.0 / (4.0 * DC * DC)
    odd = (n % 2) != 0
    h[odd] = -1.0 / (np.pi * n[odd] * DC) ** 2
    idx = (np.arange(D)[None, :] - np.arange(D)[:, None]) % PAD
    return (h[idx] * (12.0 * DC * np.pi / A)).astype(np.float32)


def _k_eff():
    c = -20.0 + (np.arange(N) + 0.5) * CELL
    X, Y = np.meshgrid(c, c, indexing="ij")
    th = (np.arange(A) + 0.5) * np.pi / A
    t = np.cos(th)[:, None] * X.ravel()[None, :] + np.sin(th)[:, None] * Y.ravel()[None, :]
    k = (t - (-RHO + 0.5 * DC)) / DC
    k0 = np.clip(np.floor(k), 0, D - 2)
    w = np.clip(k - k0, 0.0, 1.0)
    return (k0 + w).astype(np.float32).reshape(A, N, N)


_DEV = None
_DEV_FAILED = False


def _init_device():
    global _DEV, _DEV_FAILED
    if _DEV is not None:
        return _DEV
    if _DEV_FAILED:
        raise RuntimeError("device init previously failed")

    import jax
    import jax.numpy as jnp
    import ml_dtypes
    from functools import partial
    import concourse.bass as bass  # noqa: F401  (ensures concourse importable)
    import concourse.mybir as mybir
    from concourse.tile import TileContext
    from concourse.bass2jax import bass_jit, bass_shard_map
    from concourse.masks import make_identity
    from jax.sharding import Mesh, NamedSharding, PartitionSpec as P
    from jax.experimental.shard_map import shard_map

    BF = mybir.dt.bfloat16
    F32d = mybir.dt.float32

    F = _filter_matrix()
    KE = _k_eff()
    kg_host = np.stack([
        np.concatenate([KE[0:NPAIR, 8 * c:8 * c + 8, :],
                        KE[0:NPAIR, 120 - 8 * c:128 - 8 * c, :]], axis=1)
        for c in range(8)])                    # [8, 142, 16, 128]
    ksp_host = KE[142, 0, :].copy()            # [128]

    devs = jax.devices()[:8]
    assert len(devs) == 8
    mesh = Mesh(np.array(devs), ("core",))
    shard0 = NamedSharding(mesh, P("core"))

    @partial(jax.jit, out_shardings=(shard0,) * 3)
    def gen_w(kg, Fd, ksp):
        def body(kl, Fd, ksp):
            j = jnp.arange(D, dtype=jnp.float32)
            klf = kl.reshape(NPAIR, 16 * 128)
            Wblk = jax.nn.relu(1.0 - jnp.abs(j[None, :, None] - klf[:, None, :]))
            Wt = jnp.einsum("dj,tjq->tdq", Fd, Wblk)        # [142,183,2048]
            w0 = Wt[:, 0:128, :].reshape(NPAIR, 128, 16, 128).astype(jnp.bfloat16)
            w1 = Wt[:, 128:183, :].reshape(71, 110, 16, 128).astype(jnp.bfloat16)
            Wsp = jax.nn.relu(1.0 - jnp.abs(j[:, None] - ksp[None, :]))
            ws = (Fd @ Wsp).astype(jnp.bfloat16)
            return w0, w1, ws
        return shard_map(body, mesh=mesh, in_specs=(P("core"), P(), P()),
                         out_specs=(P("core"),) * 3, check_rep=False)(kg, Fd, ksp)

    I8 = mybir.dt.int8

    @bass_jit(num_devices=8)
    def fbp_bass(nc, xs, w0, w1, ws):
        # xs [285,183,32]: this core's batch slice, detector-major (host-packed)
        wire_dt = I8 if xs.dtype == I8 else BF
        out = nc.dram_tensor("out", [256, 16, 128], BF, kind="ExternalOutput")
        with TileContext(nc) as tc:
            with tc.tile_pool(name="wp", bufs=3) as wp, \
                 tc.tile_pool(name="rp", bufs=3) as rp, \
                 tc.tile_pool(name="ri", bufs=3) as ri, \
                 tc.tile_pool(name="ps", bufs=8, space="PSUM") as ps, \
                 tc.tile_pool(name="sp", bufs=1) as sp, \
                 tc.tile_pool(name="dp", bufs=1, space="DRAM") as dp:

                # in-kernel AllGather of the quantized sinogram (one dispatch
                # for the whole per-call pipeline)
                bounce_in = dp.tile([285, 183, 32], wire_dt, tag="bin")
                bounce_out = dp.tile([8, 285, 183, 32], wire_dt, tag="bout")
                xt8 = dp.tile([285, 183, 256], wire_dt, tag="xt8")
                nc.sync.dma_start(out=bounce_in, in_=xs[:, :, :])
                nc.gpsimd.collective_compute(
                    "AllGather", mybir.AluOpType.bypass,
                    replica_groups=[list(range(8))],
                    ins=[bounce_in], outs=[bounce_out])
                for c in range(8):
                    nc.sync.dma_start(out=xt8[:, :, 32 * c:32 * c + 32],
                                      in_=bounce_out[c])

                ident = sp.tile([128, 128], BF, tag="ident")
                make_identity(nc, ident)

                sw0 = sp.tile([128, 128], BF, tag="sw0")
                sw1 = sp.tile([55, 128], BF, tag="sw1")
                nc.sync.dma_start(out=sw0, in_=ws[0:128, :])
                nc.sync.dma_start(out=sw1, in_=ws[128:183, :])
                xs0i = sp.tile([128, 256], wire_dt, tag="xs0i")
                xs1i = sp.tile([55, 256], wire_dt, tag="xs1i")
                nc.sync.dma_start(out=xs0i, in_=xt8[142, 0:128, :])
                nc.sync.dma_start(out=xs1i, in_=xt8[142, 128:183, :])
                xs0 = sp.tile([128, 256], BF, tag="xs0")
                xs1 = sp.tile([55, 256], BF, tag="xs1")
                nc.vector.tensor_copy(xs0, xs0i)
                nc.vector.tensor_copy(xs1, xs1i)
                pspec = ps.tile([128, 256], F32d, tag="acc")
                nc.tensor.matmul(pspec, lhsT=sw0, rhs=xs0, start=True, stop=False)
                nc.tensor.matmul(pspec, lhsT=sw1, rhs=xs1, start=False, stop=True)
                special = sp.tile([128, 256], F32d, tag="special")
                nc.vector.tensor_copy(special, pspec)

                stg0 = sp.tile([128, 16, 128], BF, tag="stg0")
                stg1 = sp.tile([128, 16, 128], BF, tag="stg1")
                stgs = (stg0, stg1)

                for sw in range(2):
                    accs = [ps.tile([128, 512], F32d, tag="acc", name=f"acc{sw}_{r}")
                            for r in range(8)]
                    for t in range(NPAIR):
                        a, ast = t, 284 - t
                        r0i = ri.tile([128, 512], wire_dt, tag="r0i", name=f"r0i_{sw}_{t}")
                        nc.sync.dma_start(out=r0i[:, 0:256], in_=xt8[a, 0:128, :])
                        nc.sync.dma_start(out=r0i[:, 256:512], in_=xt8[ast, 0:128, :])
                        r0 = rp.tile([128, 512], BF, tag="r0", name=f"r0_{sw}_{t}")
                        nc.vector.tensor_copy(r0, r0i)
                        w0t = wp.tile([128, 8, 128], BF, tag="w0", name=f"w0_{sw}_{t}")
                        nc.sync.dma_start(out=w0t, in_=w0[t, :, 8 * sw:8 * sw + 8, :])
                        for r in range(8):
                            nc.tensor.matmul(accs[r], lhsT=w0t[:, r, :], rhs=r0,
                                             start=(t == 0), stop=False)
                        if t % 2 == 1:
                            tt = t // 2
                            r1i = ri.tile([110, 512], wire_dt, tag="r1i", name=f"r1i_{sw}_{t}")
                            for kk, aa in enumerate((t - 1, t)):
                                nc.sync.dma_start(out=r1i[55 * kk:55 * kk + 55, 0:256],
                                                  in_=xt8[aa, 128:183, :])
                                nc.sync.dma_start(out=r1i[55 * kk:55 * kk + 55, 256:512],
                                                  in_=xt8[284 - aa, 128:183, :])
                            r1 = rp.tile([110, 512], BF, tag="r1", name=f"r1_{sw}_{t}")
                            nc.vector.tensor_copy(r1, r1i)
                            w1t = wp.tile([110, 8, 128], BF, tag="w1", name=f"w1_{sw}_{t}")
                            nc.sync.dma_start(out=w1t, in_=w1[tt, :, 8 * sw:8 * sw + 8, :])
                            last = (t == NPAIR - 1)
                            for r in range(8):
                                nc.tensor.matmul(accs[r], lhsT=w1t[:, r, :], rhs=r1,
                                                 start=False, stop=last)

                    for r in range(8):
                        sb = rp.tile([128, 512], BF, tag="sb", name=f"sb{sw}_{r}")
                        if sw == 0:
                            nc.vector.tensor_add(sb[:, 0:256], accs[r][:, 0:256], special)
                            nc.vector.tensor_add(sb[:, 256:512], accs[r][:, 256:512], special)
                        else:
                            nc.vector.tensor_copy(sb, accs[r])
                        for piece in range(4):
                            half = piece % 2
                            flip = piece // 2
                            if sw == 0:
                                slot = r if flip == 0 else 15 - r
                            else:
                                slot = 8 + r if flip == 0 else 7 - r
                            tp = ps.tile([128, 128], BF, tag="acc",
                                         name=f"tp{sw}_{r}_{piece}")
                            nc.tensor.transpose(tp, in_=sb[:, piece * 128:(piece + 1) * 128],
                                                identity=ident)
                            if sw == 0:
                                nc.vector.tensor_copy(stgs[half][:, slot, :], tp)
                            else:
                                nc.vector.tensor_add(stgs[half][:, slot, :],
                                                     stgs[half][:, slot, :], tp)
                nc.sync.dma_start(out=out[0:128, :, :], in_=stg0)
                nc.sync.dma_start(out=out[128:256, :, :], in_=stg1)
        return out

    fbp_dev = bass_shard_map(fbp_bass, mesh=mesh,
                             in_specs=(P("core"),) * 4, out_specs=P("core"))

    try:
        W0g, W1g, Wsg = gen_w(kg_host.reshape(8 * NPAIR, 16, 128), F, ksp_host)
        for t_ in (W0g, W1g, Wsg):
            t_.block_until_ready()
        # compile the int8-wire NEFF now so the first real call is fast
        dummy = np.zeros((8 * A, D, 32), np.int8)
        o0 = fbp_dev(dummy, W0g, W1g, Wsg)
        o0.block_until_ready()
    except Exception:
        _DEV_FAILED = True
        raise

    _DEV = dict(jax=jax, np_bf16=np.dtype(ml_dtypes.bfloat16),
                fbp=fbp_dev, W=(W0g, W1g, Wsg), in_sharding=shard0)
    return _DEV


def _assemble(on, scale):
    # on: [2048, 16, 128] bf16-ish array from device (8 core shards stacked)
    on = np.asarray(on).astype(np.float32).reshape(8, B, 16, 128)
    if scale != 1.0:
        on *= scale
    img = np.empty((B, N, N), np.float32)
    for c in range(8):
        img[:, 8 * c:8 * c + 8, :] = on[c][:, 0:8, :]
        img[:, 120 - 8 * c:128 - 8 * c, :] = on[c][:, 8:16, :]
    return img


def _pack_wire(xq):
    # [256,285,183] -> per-core detector-major slices stacked: [8*285,183,32]
    return np.ascontiguousarray(
        xq.reshape(8, 32, A, D).transpose(0, 2, 3, 1)).reshape(8 * A, D, 32)


def _kernel_device(x):
    st = _init_device()
    absmax = float(np.abs(x).max())
    if absmax <= INT8_ABSMAX:
        tmp = x * INT8_SCALE
        np.rint(tmp, out=tmp)
        wire = _pack_wire(tmp.astype(np.int8))
        scale = 1.0 / INT8_SCALE
    else:
        wire = _pack_wire(x.astype(st["np_bf16"]))
        scale = 1.0
    o = st["fbp"](wire, *st["W"])
    return _assemble(o, scale)


# ---------------- host fallback (exact float32 semantics) ----------------

_HOST = None


def _init_host():
    global _HOST
    if _HOST is not None:
        return _HOST
    F = _filter_matrix()
    KE = _k_eff().reshape(A, N * N)
    k0 = np.clip(np.floor(KE), 0, D - 2).astype(np.int32)
    w = (KE - k0).astype(np.float32)
    gi = (k0 + (np.arange(A, dtype=np.int64) * D)[:, None]).astype(np.int32)
    bp = None
    try:
        import numba

        @numba.njit(fastmath=True, cache=True)
        def bp_(qT, giT, wT, out):
            Pn, nA = giT.shape
            Bc = qT.shape[1]
            acc = np.empty(Bc, np.float32)
            for p in range(Pn):
                r = giT[p, 0]
                w1 = wT[p, 0]
                w0 = np.float32(1.0) - w1
                for ci in range(Bc):
                    acc[ci] = w0 * qT[r, ci] + w1 * qT[r + 1, ci]
                for tt in range(1, nA):
                    r = giT[p, tt]
                    w1 = wT[p, tt]
                    w0 = np.float32(1.0) - w1
                    for ci in range(Bc):
                        acc[ci] += w0 * qT[r, ci] + w1 * qT[r + 1, ci]
                out[p, :] = acc

        bp = bp_
    except Exception:
        bp = None
    _HOST = dict(F=F, k0=k0, w=w, giT=np.ascontiguousarray(gi.T),
                 wT=np.ascontiguousarray(w.T), bp=bp)
    return _HOST


def _kernel_host(x):
    st = _init_host()
    b = x.shape[0]
    q = (x.reshape(b * A, D) @ st["F"]).reshape(b, A * D)
    if st["bp"] is not None:
        qT = np.ascontiguousarray(q.T)
        out = np.empty((N * N, b), np.float32)
        st["bp"](qT, st["giT"], st["wT"], out)
        return np.ascontiguousarray(out.T).reshape(b, N, N)
    out = np.zeros((b, N * N), np.float32)
    q3 = q.reshape(b, A, D)
    for a in range(A):
        qa = q3[:, a, :]
        i0 = st["k0"][a]
        wa = st["w"][a]
        out += (1.0 - wa) * qa[:, i0] + wa * qa[:, i0 + 1]
    return out.reshape(b, N, N)


def kernel(x: np.ndarray) -> np.ndarray:
    x = np.asarray(x, dtype=np.float32)
    for _attempt in range(2):  # one retry absorbs transient device resets
        try:
            return _kernel_device(x)
        except Exception:
            continue
    return _kernel_host(x)


if __name__ == "__main__":
    rng = np.random.default_rng(0)
    x = rng.standard_normal((B, A, D), dtype=np.float32)
    y = kernel(x)
    print(y.shape, y.dtype, float(np.abs(y).max()))


# revision 6
# speedup vs baseline: 1.3570x; 1.3570x over previous
"""Backward_projection (FBP: ramp filter + backprojection) on 8 trn2 NeuronCores.

Device formulation (hand-written Bass/Tile kernel, one NEFF per core, SPMD):
  * The ramp filter (exact 183x183 circulant-section matmul, all scalar
    factors folded) is FOLDED into the backprojection matrix:
    out[b, p] = sum_d x[b, a, d] * Wt_a[d, p],  Wt_a = F @ tri(d - k[a, p]),
    so the whole FBP is one accumulation of 285 per-angle matmuls on the PE.
  * Angle-flip pairing: k_{284-a}(127-i, j) == k_a(i, j) exactly, so the
    weight tile of (angle a, image row i) also serves (284-a, 127-i); each
    matmul streams N=512 (256 batches x 2 pair-halves) through one
    stationary tile.  Angle 142 (theta = pi/2) is row-independent and is
    computed once and added in the epilogue.
  * K = 183 splits into chunks of 128 + 55; the 55-chunks of two adjacent
    pairs are packed into one [110, .] stationary tile (25% fewer streams).
  * Sharding: each core owns the symmetric image-row set
    [8c, 8c+8) u [120-8c, 128-8c) (16 rows, 2048 pixels) for all 256
    batches, accumulated over two 8-row PSUM sweeps.  x is uploaded
    batch-sharded (contiguous slices) and all-gathered on device.
  * Weights (~107MB/core bf16) are GENERATED ON DEVICE once from a small
    geometry table and stay resident; per call only the quantized sinogram
    crosses the (slow) host link and the bf16 image returns.

The wire uses int8 (validated against the reference: quantization noise
contributes ~1e-2 of the 2e-2 absmax-relative budget; the bf16 path at
~5e-3 is kept as an automatic fallback for inputs exceeding the int8
range).  Falls back to a host numba/numpy implementation if the device
path is unavailable.
"""

import numpy as np

# --- geometry constants (parallel_beam_geometry on a 128^2 grid) ---
N = 128
CELL = 40.0 / N
RHO = float(np.sqrt(2.0) * 20.0)
A = 285
D = 183
DC = 2.0 * RHO / D
PAD = 512
B = 256
NPAIR = 142
INT8_SCALE = 127.0 / 5.5
INT8_ABSMAX = 5.45  # inputs beyond this use the bf16 wire


def _filter_matrix():
    n = (np.fft.fftfreq(PAD) * PAD).astype(np.int64)
    h = np.zeros(PAD, np.float64)
    h[0] = 1.0 / (4.0 * DC * DC)
    odd = (n % 2) != 0
    h[odd] = -1.0 / (np.pi * n[odd] * DC) ** 2
    idx = (np.arange(D)[None, :] - np.arange(D)[:, None]) % PAD
    return (h[idx] * (12.0 * DC * np.pi / A)).astype(np.float32)


def _k_eff():
    c = -20.0 + (np.arange(N) + 0.5) * CELL
    X, Y = np.meshgrid(c, c, indexing="ij")
    th = (np.arange(A) + 0.5) * np.pi / A
    t = np.cos(th)[:, None] * X.ravel()[None, :] + np.sin(th)[:, None] * Y.ravel()[None, :]
    k = (t - (-RHO + 0.5 * DC)) / DC
    k0 = np.clip(np.floor(k), 0, D - 2)
    w = np.clip(k - k0, 0.0, 1.0)
    return (k0 + w).astype(np.float32).reshape(A, N, N)


_DEV = None
_DEV_FAILED = False


def _init_device():
    global _DEV, _DEV_FAILED
    if _DEV is not None:
        return _DEV
    if _DEV_FAILED:
        raise RuntimeError("device init previously failed")

    import jax
    import jax.numpy as jnp
    import ml_dtypes
    from functools import partial
    import concourse.bass as bass  # noqa: F401  (ensures concourse importable)
    import concourse.mybir as mybir
    from concourse.tile import TileContext
    from concourse.bass2jax import bass_jit, bass_shard_map
    from concourse.masks import make_identity
    from jax.sharding import Mesh, NamedSharding, PartitionSpec as P
    from jax.experimental.shard_map import shard_map

    BF = mybir.dt.bfloat16
    F32d = mybir.dt.float32

    F = _filter_matrix()
    KE = _k_eff()
    kg_host = np.stack([
        np.concatenate([KE[0:NPAIR, 8 * c:8 * c + 8, :],
                        KE[0:NPAIR, 120 - 8 * c:128 - 8 * c, :]], axis=1)
        for c in range(8)])                    # [8, 142, 16, 128]
    ksp_host = KE[142, 0, :].copy()            # [128]

    devs = jax.devices()[:8]
    assert len(devs) == 8
    mesh = Mesh(np.array(devs), ("core",))
    shard0 = NamedSharding(mesh, P("core"))

    @partial(jax.jit, out_shardings=(shard0,) * 3)
    def gen_w(kg, Fd, ksp):
        def body(kl, Fd, ksp):
            j = jnp.arange(D, dtype=jnp.float32)
            klf = kl.reshape(NPAIR, 16 * 128)
            Wblk = jax.nn.relu(1.0 - jnp.abs(j[None, :, None] - klf[:, None, :]))
            Wt = jnp.einsum("dj,tjq->tdq", Fd, Wblk)        # [142,183,2048]
            w0 = Wt[:, 0:128, :].reshape(NPAIR, 128, 16, 128).astype(jnp.bfloat16)
            w1 = Wt[:, 128:183, :].reshape(71, 110, 16, 128).astype(jnp.bfloat16)
            Wsp = jax.nn.relu(1.0 - jnp.abs(j[:, None] - ksp[None, :]))
            ws = (Fd @ Wsp).astype(jnp.bfloat16)
            return w0, w1, ws
        return shard_map(body, mesh=mesh, in_specs=(P("core"), P(), P()),
                         out_specs=(P("core"),) * 3, check_rep=False)(kg, Fd, ksp)

    I8 = mybir.dt.int8

    @bass_jit(num_devices=8)
    def fbp_bass(nc, xs, w0, w1, ws):
        # xs [285,183,32]: this core's batch slice, detector-major (host-packed)
        wire_dt = I8 if xs.dtype == I8 else BF
        out = nc.dram_tensor("out", [256, 16, 128], BF, kind="ExternalOutput")
        with TileContext(nc) as tc:
            with tc.tile_pool(name="wp", bufs=3) as wp, \
                 tc.tile_pool(name="rp", bufs=3) as rp, \
                 tc.tile_pool(name="ri", bufs=3) as ri, \
                 tc.tile_pool(name="ps", bufs=8, space="PSUM") as ps, \
                 tc.tile_pool(name="sp", bufs=1) as sp, \
                 tc.tile_pool(name="dp", bufs=1, space="DRAM") as dp:

                # in-kernel AllGather of the quantized sinogram (one dispatch
                # for the whole per-call pipeline)
                bounce_in = dp.tile([285, 183, 32], wire_dt, tag="bin")
                bounce_out = dp.tile([8, 285, 183, 32], wire_dt, tag="bout")
                xt8 = dp.tile([285, 183, 256], wire_dt, tag="xt8")
                nc.sync.dma_start(out=bounce_in, in_=xs[:, :, :])
                nc.gpsimd.collective_compute(
                    "AllGather", mybir.AluOpType.bypass,
                    replica_groups=[list(range(8))],
                    ins=[bounce_in], outs=[bounce_out])
                for c in range(8):
                    nc.sync.dma_start(out=xt8[:, :, 32 * c:32 * c + 32],
                                      in_=bounce_out[c])

                ident = sp.tile([128, 128], BF, tag="ident")
                make_identity(nc, ident)

                sw0 = sp.tile([128, 128], BF, tag="sw0")
                sw1 = sp.tile([55, 128], BF, tag="sw1")
                nc.sync.dma_start(out=sw0, in_=ws[0:128, :])
                nc.sync.dma_start(out=sw1, in_=ws[128:183, :])
                xs0i = sp.tile([128, 256], wire_dt, tag="xs0i")
                xs1i = sp.tile([55, 256], wire_dt, tag="xs1i")
                nc.sync.dma_start(out=xs0i, in_=xt8[142, 0:128, :])
                nc.sync.dma_start(out=xs1i, in_=xt8[142, 128:183, :])
                xs0 = sp.tile([128, 256], BF, tag="xs0")
                xs1 = sp.tile([55, 256], BF, tag="xs1")
                nc.vector.tensor_copy(xs0, xs0i)
                nc.vector.tensor_copy(xs1, xs1i)
                pspec = ps.tile([128, 256], F32d, tag="acc")
                nc.tensor.matmul(pspec, lhsT=sw0, rhs=xs0, start=True, stop=False)
                nc.tensor.matmul(pspec, lhsT=sw1, rhs=xs1, start=False, stop=True)
                special = sp.tile([128, 256], F32d, tag="special")
                nc.vector.tensor_copy(special, pspec)

                stg0 = sp.tile([128, 16, 128], BF, tag="stg0")
                stg1 = sp.tile([128, 16, 128], BF, tag="stg1")
                stgs = (stg0, stg1)

                for sw in range(2):
                    accs = [ps.tile([128, 512], F32d, tag="acc", name=f"acc{sw}_{r}")
                            for r in range(8)]
                    for t in range(NPAIR):
                        a, ast = t, 284 - t
                        r0i = ri.tile([128, 512], wire_dt, tag="r0i", name=f"r0i_{sw}_{t}")
                        nc.sync.dma_start(out=r0i[:, 0:256], in_=xt8[a, 0:128, :])
                        nc.sync.dma_start(out=r0i[:, 256:512], in_=xt8[ast, 0:128, :])
                        r0 = rp.tile([128, 512], BF, tag="r0", name=f"r0_{sw}_{t}")
                        nc.vector.tensor_copy(r0, r0i)
                        w0t = wp.tile([128, 8, 128], BF, tag="w0", name=f"w0_{sw}_{t}")
                        nc.sync.dma_start(out=w0t, in_=w0[t, :, 8 * sw:8 * sw + 8, :])
                        for r in range(8):
                            nc.tensor.matmul(accs[r], lhsT=w0t[:, r, :], rhs=r0,
                                             start=(t == 0), stop=False)
                        if t % 2 == 1:
                            tt = t // 2
                            r1i = ri.tile([110, 512], wire_dt, tag="r1i", name=f"r1i_{sw}_{t}")
                            for kk, aa in enumerate((t - 1, t)):
                                nc.sync.dma_start(out=r1i[55 * kk:55 * kk + 55, 0:256],
                                                  in_=xt8[aa, 128:183, :])
                                nc.sync.dma_start(out=r1i[55 * kk:55 * kk + 55, 256:512],
                                                  in_=xt8[284 - aa, 128:183, :])
                            r1 = rp.tile([110, 512], BF, tag="r1", name=f"r1_{sw}_{t}")
                            nc.vector.tensor_copy(r1, r1i)
                            w1t = wp.tile([110, 8, 128], BF, tag="w1", name=f"w1_{sw}_{t}")
                            nc.sync.dma_start(out=w1t, in_=w1[tt, :, 8 * sw:8 * sw + 8, :])
                            last = (t == NPAIR - 1)
                            for r in range(8):
                                nc.tensor.matmul(accs[r], lhsT=w1t[:, r, :], rhs=r1,
                                                 start=False, stop=last)

                    for r in range(8):
                        sb = rp.tile([128, 512], BF, tag="sb", name=f"sb{sw}_{r}")
                        if sw == 0:
                            nc.vector.tensor_add(sb[:, 0:256], accs[r][:, 0:256], special)
                            nc.vector.tensor_add(sb[:, 256:512], accs[r][:, 256:512], special)
                        else:
                            nc.vector.tensor_copy(sb, accs[r])
                        for piece in range(4):
                            half = piece % 2
                            flip = piece // 2
                            if sw == 0:
                                slot = r if flip == 0 else 15 - r
                            else:
                                slot = 8 + r if flip == 0 else 7 - r
                            tp = ps.tile([128, 128], BF, tag="acc",
                                         name=f"tp{sw}_{r}_{piece}")
                            nc.tensor.transpose(tp, in_=sb[:, piece * 128:(piece + 1) * 128],
                                                identity=ident)
                            if sw == 0:
                                nc.vector.tensor_copy(stgs[half][:, slot, :], tp)
                            else:
                                nc.vector.tensor_add(stgs[half][:, slot, :],
                                                     stgs[half][:, slot, :], tp)
                nc.sync.dma_start(out=out[0:128, :, :], in_=stg0)
                nc.sync.dma_start(out=out[128:256, :, :], in_=stg1)
        return out

    fbp_dev = bass_shard_map(fbp_bass, mesh=mesh,
                             in_specs=(P("core"),) * 4, out_specs=P("core"))

    try:
        W0g, W1g, Wsg = gen_w(kg_host.reshape(8 * NPAIR, 16, 128), F, ksp_host)
        for t_ in (W0g, W1g, Wsg):
            t_.block_until_ready()
        # compile the int8-wire NEFF now so the first real call is fast
        dummy = np.zeros((8 * A, D, 32), np.int8)
        o0 = fbp_dev(dummy, W0g, W1g, Wsg)
        o0.block_until_ready()
    except Exception:
        _DEV_FAILED = True
        raise

    _DEV = dict(jax=jax, np_bf16=np.dtype(ml_dtypes.bfloat16),
                fbp=fbp_dev, W=(W0g, W1g, Wsg), in_sharding=shard0)
    return _DEV


def _assemble(on, scale):
    # on: [2048, 16, 128] bf16-ish array from device (8 core shards stacked);
    # fuse the dequant scale into the bf16->f32 cast while reassembling rows
    on = np.asarray(on).reshape(8, B, 16, 128)
    img = np.empty((B, N, N), np.float32)
    for c in range(8):
        np.multiply(on[c][:, 0:8, :], scale, out=img[:, 8 * c:8 * c + 8, :],
                    dtype=np.float32, casting="unsafe")
        np.multiply(on[c][:, 8:16, :], scale, out=img[:, 120 - 8 * c:128 - 8 * c, :],
                    dtype=np.float32, casting="unsafe")
    return img


def _pack_wire(xq):
    # [256,285,183] -> per-core detector-major slices stacked: [8*285,183,32]
    return np.ascontiguousarray(
        xq.reshape(8, 32, A, D).transpose(0, 2, 3, 1)).reshape(8 * A, D, 32)


_QBUFS = None


def _quant_pack_int8(x):
    # one fused quantize pass into preallocated buffers, then pack
    global _QBUFS
    if _QBUFS is None:
        _QBUFS = (np.empty(x.shape, np.float32), np.empty(x.shape, np.int8))
    tmp, qi = _QBUFS
    np.multiply(x, INT8_SCALE, out=tmp)
    np.rint(tmp, out=tmp)
    np.copyto(qi, tmp, casting="unsafe")
    return _pack_wire(qi)


def _kernel_device(x):
    st = _init_device()
    absmax = float(np.abs(x).max())
    if absmax <= INT8_ABSMAX:
        wire = _quant_pack_int8(x)
        scale = 1.0 / INT8_SCALE
    else:
        wire = _pack_wire(x.astype(st["np_bf16"]))
        scale = 1.0
    o = st["fbp"](wire, *st["W"])
    return _assemble(o, scale)


# ---------------- host fallback (exact float32 semantics) ----------------

_HOST = None


def _init_host():
    global _HOST
    if _HOST is not None:
        return _HOST
    F = _filter_matrix()
    KE = _k_eff().reshape(A, N * N)
    k0 = np.clip(np.floor(KE), 0, D - 2).astype(np.int32)
    w = (KE - k0).astype(np.float32)
    gi = (k0 + (np.arange(A, dtype=np.int64) * D)[:, None]).astype(np.int32)
    bp = None
    try:
        import numba

        @numba.njit(fastmath=True, cache=True)
        def bp_(qT, giT, wT, out):
            Pn, nA = giT.shape
            Bc = qT.shape[1]
            acc = np.empty(Bc, np.float32)
            for p in range(Pn):
                r = giT[p, 0]
                w1 = wT[p, 0]
                w0 = np.float32(1.0) - w1
                for ci in range(Bc):
                    acc[ci] = w0 * qT[r, ci] + w1 * qT[r + 1, ci]
                for tt in range(1, nA):
                    r = giT[p, tt]
                    w1 = wT[p, tt]
                    w0 = np.float32(1.0) - w1
                    for ci in range(Bc):
                        acc[ci] += w0 * qT[r, ci] + w1 * qT[r + 1, ci]
                out[p, :] = acc

        bp = bp_
    except Exception:
        bp = None
    _HOST = dict(F=F, k0=k0, w=w, giT=np.ascontiguousarray(gi.T),
                 wT=np.ascontiguousarray(w.T), bp=bp)
    return _HOST


def _kernel_host(x):
    st = _init_host()
    b = x.shape[0]
    q = (x.reshape(b * A, D) @ st["F"]).reshape(b, A * D)
    if st["bp"] is not None:
        qT = np.ascontiguousarray(q.T)
        out = np.empty((N * N, b), np.float32)
        st["bp"](qT, st["giT"], st["wT"], out)
        return np.ascontiguousarray(out.T).reshape(b, N, N)
    out = np.zeros((b, N * N), np.float32)
    q3 = q.reshape(b, A, D)
    for a in range(A):
        qa = q3[:, a, :]
        i0 = st["k0"][a]
        wa = st["w"][a]
        out += (1.0 - wa) * qa[:, i0] + wa * qa[:, i0 + 1]
    return out.reshape(b, N, N)


def kernel(x: np.ndarray) -> np.ndarray:
    x = np.asarray(x, dtype=np.float32)
    for _attempt in range(2):  # one retry absorbs transient device resets
        try:
            return _kernel_device(x)
        except Exception:
            continue
    return _kernel_host(x)


if __name__ == "__main__":
    rng = np.random.default_rng(0)
    x = rng.standard_normal((B, A, D), dtype=np.float32)
    y = kernel(x)
    print(y.shape, y.dtype, float(np.abs(y).max()))
